# revision 1
# baseline (speedup 1.0000x reference)
"""Cached grouped-query multi-head attention on 8 Trainium2 cores.

Sharding: core c -> batch b = c//2, head-half = c%2 (8 of 16 heads, 2 of 4
KV groups per core). Wq/Wk column-parallel, Wo row-parallel; the two
partial Wo products per batch are summed on the host (the "all-reduce").

Device kernel (per core, fp32 data / float32r matmuls):
  x^T via PE transposes -> Q^T/K^T/V projections -> RoPE (head-dim stored
  even-dims-then-odd-dims so rotation halves are partition-contiguous;
  host permutes Wq/Wk columns accordingly) -> scores computed transposed
  [k, q] so softmax probs are already PV-ready -> exp (no max subtraction;
  scores are O(1)) -> multiplicative mask on partial tiles only ->
  PV (out^T layout) + all-ones matmul for the softmax denominator ->
  reciprocal scale -> row-parallel Wo -> partial [LQ, D] output.
"""

import math
import sys

import numpy as np

sys.path.insert(0, "/opt/trn_rl_repo")

B, LQ, D = 4, 1024, 2048
H, G = 16, 4
HD = 128            # head dim
GS = H // G         # heads per group
PAST = 1024
LK = PAST + LQ      # 2048
NCORES = 8
NH = 8              # local heads per core
NG = 2              # local groups per core
KSUB = D // 128     # 16 contraction subtiles over D
QC = LQ // 512      # 2 query chunks of 512
QS = LQ // 128      # 8 query subtiles of 128
KC = LK // 128      # 16 key chunks of 128
NCH = D // 512      # 4 output column chunks

_PERM = np.concatenate([np.arange(0, HD, 2), np.arange(1, HD, 2)])
_PROG_CACHE = {}


ATTN_BF16 = True  # bf16 scores/PV/den (2x LDW pipelining, ~3e-3 rel err)


def _build_program(classes, n_part, attn_bf16=False):
    """Build the per-core Bass/Tile program.

    classes[(qc, kc)] = ("full"|"skip"|"part", partial_idx_or_None),
    identical on every core (the mask is shared).
    """
    import concourse.bacc as bacc
    import concourse.mybir as mybir
    import concourse.tile as tile
    from concourse.masks import make_identity

    f32 = mybir.dt.float32
    f32r = mybir.dt.float32r
    adt = mybir.dt.bfloat16 if attn_bf16 else f32r
    AF = mybir.ActivationFunctionType
    OP = mybir.AluOpType

    nc = bacc.Bacc("TRN2", target_bir_lowering=False, debug=False,
                   num_devices=NCORES)

    x_d = nc.dram_tensor("x", [LQ, D], f32, kind="ExternalInput").ap()
    wq_d = nc.dram_tensor("wq", [D, NH * HD], f32r, kind="ExternalInput").ap()
    bq_d = nc.dram_tensor("bq", [NH, HD, 1], f32, kind="ExternalInput").ap()
    wk_d = nc.dram_tensor("wk", [D, NG * HD], f32r, kind="ExternalInput").ap()
    bk_d = nc.dram_tensor("bk", [NG, HD, 1], f32, kind="ExternalInput").ap()
    wv_d = nc.dram_tensor("wv", [D, NG * HD], f32r, kind="ExternalInput").ap()
    bv_d = nc.dram_tensor("bv", [1, NG * HD], f32, kind="ExternalInput").ap()
    pk_d = nc.dram_tensor("pk", [NG, PAST, HD], f32, kind="ExternalInput").ap()
    pv_d = nc.dram_tensor("pv", [NG, PAST, HD], adt, kind="ExternalInput").ap()
    rot_d = nc.dram_tensor("rot", [LQ, HD // 2], f32, kind="ExternalInput").ap()
    wo_d = nc.dram_tensor("wo", [NH * HD, D], f32r, kind="ExternalInput").ap()
    bo_d = nc.dram_tensor("bo", [1, D], f32, kind="ExternalInput").ap()
    mp_d = None
    if n_part:
        mp_d = nc.dram_tensor("maskp", [n_part, 128, 512], adt,
                              kind="ExternalInput").ap()
    out_d = nc.dram_tensor("out", [LQ, D], f32, kind="ExternalOutput").ap()

    # active key chunks per query chunk: list of (kc, partial_idx|None)
    active = {qc: [(kc, classes[(qc, kc)][1])
                   for kc in range(KC) if classes[(qc, kc)][0] != "skip"]
              for qc in range(QC)}

    scl = 1.0 / math.sqrt(HD)

    with tile.TileContext(nc) as tc:
        with (
            tc.tile_pool(name="const", bufs=1) as const,
            tc.tile_pool(name="persist", bufs=1) as persist,
            tc.tile_pool(name="raw", bufs=2) as raw,
            tc.tile_pool(name="ropet", bufs=1) as ropetp,
        ):
            ident = const.tile([128, 128], f32)
            make_identity(nc, ident)
            ones_f = const.tile([128, 128], f32)
            nc.gpsimd.memset(ones_f, 1.0)
            ones_mat = const.tile([128, 128], adt)
            nc.vector.tensor_copy(ones_mat, ones_f)

            bias_qk = const.tile([128, NH + NG], f32)
            for h in range(NH):
                nc.sync.dma_start(bias_qk[:, h:h + 1], bq_d[h])
            for g in range(NG):
                nc.sync.dma_start(bias_qk[:, NH + g:NH + g + 1], bk_d[g])

            QT = persist.tile([128, NH, LQ], adt)     # roped Q^T (perm rows)
            KT = persist.tile([128, NG, LK], adt)     # K^T cache (perm rows)
            V = [persist.tile([128, KC, HD], adt, tag=f"v{g}", name=f"v{g}")
                 for g in range(NG)]

            # full-height rotary tables: rows 0:64 and 64:128 both hold the
            # 64 frequencies; ssgnF carries -sin on top, +sin on bottom, so
            #   roped = src*cosF + swap(src)*ssgnF
            # where swap exchanges the two partition halves (x1<->x2):
            #   top: x1*cos + x2*(-sin)   bot: x2*cos + x1*(+sin)
            cosF = const.tile([128, LQ], f32)
            ssgnF = const.tile([128, LQ], f32)

            def rope(src, dst):
                # src/dst [128, LQ]; rows 0:64 = even dims, 64:128 = odd
                swp = raw.tile([128, LQ], f32, tag="raw", name="swp")
                nc.sync.dma_start(swp[0:64], src[64:128])
                nc.sync.dma_start(swp[64:128], src[0:64])
                t = ropetp.tile([128, LQ], f32, tag="ropet")
                nc.vector.tensor_mul(t, swp, ssgnF)
                nc.vector.tensor_mul(dst, src, cosF)
                nc.vector.tensor_tensor(dst, dst, t, OP.add)

            # ---- phase 1: rotary tables + x^T ----
            with (
                tc.tile_pool(name="xt", bufs=1) as xtp,
                tc.tile_pool(name="pstp", bufs=3, space="PSUM") as pstp,
            ):
                xT = xtp.tile([128, KSUB, LQ], f32r)
                # K/V weight tiles; DMAs are issued after the x/rot loads
                # so they don't delay the critical-path x^T build (LIFO:
                # wvp closes after the V projection, wkp after K)
                wk_cm = tc.tile_pool(name="wkp", bufs=2)
                wkp = wk_cm.__enter__()
                wkgs = [wkp.tile([128, KSUB, HD], f32r, tag="wk",
                                 name=f"wk{g}") for g in range(NG)]
                wv_cm = tc.tile_pool(name="wvp", bufs=1)
                wvp = wv_cm.__enter__()
                wvt = wvp.tile([128, KSUB, NG * HD], f32r)

                def load_kv_weights():
                    for g in range(NG):
                        nc.sync.dma_start(
                            wkgs[g],
                            wk_d.rearrange("(ko ki) m -> ki ko m", ki=128)
                            [:, :, g * HD:(g + 1) * HD])
                    nc.sync.dma_start(
                        wvt, wv_d.rearrange("(ko ki) m -> ki ko m", ki=128))
                with tc.tile_pool(name="ph1", bufs=1) as ph1:
                    # ssgnF[:, :512] and cosF[0:64] double as scratch for
                    # the rotary load/transpose; both are overwritten after
                    rall = ssgnF.rearrange("p (i f) -> p i f", f=64)
                    nc.sync.dma_start(
                        rall[:, 0:8, :],
                        rot_d.rearrange("(i p) f -> p i f", p=128))
                    rotT = cosF[0:64]
                    for i in range(8):
                        ps = pstp.tile([128, 128], f32, tag="tp")
                        nc.tensor.transpose(ps[0:64, :], rall[:, i, :], ident)
                        nc.vector.tensor_copy(rotT[:, i * 128:(i + 1) * 128],
                                              ps[0:64, :])
                    # freq in [0, 2pi); Sin on ScalarE needs [-pi, pi]:
                    #   -sin(x) = sin(x - pi);  cos(x) = 1 - 2*sin^2(x/2)
                    negpi = const.tile([64, 1], f32)
                    nc.gpsimd.memset(negpi, -math.pi)
                    nc.scalar.activation(ssgnF[0:64], rotT, AF.Sin,
                                         bias=negpi)
                    s2 = ropetp.tile([64, LQ], f32, tag="ropet",
                                     name="s2")
                    nc.scalar.activation(s2, rotT, AF.Sin, scale=0.5)
                    nc.vector.tensor_mul(s2, s2, s2)
                    nc.vector.tensor_scalar(cosF[0:64], s2, -2.0, 1.0,
                                            OP.mult, OP.add)
                    # replicate to the bottom half (sin with flipped sign)
                    nc.vector.tensor_scalar_mul(s2, ssgnF[0:64], -1.0)
                    nc.sync.dma_start(ssgnF[64:128], s2)
                    nc.sync.dma_start(cosF[64:128], cosF[0:64])

                    for i in range(QS):
                        for quart in range(4):
                            xc = ph1.tile([128, 512], f32, tag="xc",
                                          bufs=3)
                            nc.sync.dma_start(
                                xc, x_d[i * 128:(i + 1) * 128,
                                        quart * 512:(quart + 1) * 512])
                            for jj in range(4):
                                j = quart * 4 + jj
                                ps = pstp.tile([128, 128], f32, tag="tp")
                                nc.tensor.transpose(
                                    ps, xc[:, jj * 128:(jj + 1) * 128], ident)
                                nc.vector.tensor_copy(
                                    xT[:, j, i * 128:(i + 1) * 128], ps)
                            if i == 1 and quart == 3:
                                load_kv_weights()

                # ---- phase 2: projections (V, K + past KV, then Q) ----
                with tc.tile_pool(name="psproj", bufs=2,
                                  space="PSUM") as psproj:
                    # V = x @ Wv + bv  (natural [q, dv] layout)
                    if True:
                        bv_sb = const.tile([1, NG * HD], f32)
                        nc.sync.dma_start(bv_sb, bv_d)
                        bv_rep = const.tile([128, NG * HD], f32)
                        nc.gpsimd.partition_broadcast(bv_rep, bv_sb)
                        for qs in range(QS):
                            ps = psproj.tile([128, 512], f32)
                            for ko in range(KSUB):
                                nc.tensor.matmul(
                                    ps[:, :NG * HD],
                                    xT[:, ko,
                                       qs * 128:(qs + 1) * 128],
                                    wvt[:, ko, :],
                                    start=(ko == 0), stop=(ko == KSUB - 1))
                            for g in range(NG):
                                nc.vector.tensor_tensor(
                                    V[g][:, PAST // 128 + qs, :],
                                    ps[:, g * HD:(g + 1) * HD],
                                    bv_rep[:, g * HD:(g + 1) * HD], OP.add)

                    wv_cm.__exit__(None, None, None)
                    # K^T (roped) + past K^T (permuted transpose) + past V
                    if True:
                        for g in range(NG):
                            wkg = wkgs[g]
                            bkt = bias_qk[:, NH + g:NH + g + 1]
                            kraw = raw.tile([128, LQ], f32, tag="raw")
                            for qc in range(QC):
                                ps = psproj.tile([128, 512], f32)
                                for ko in range(KSUB):
                                    nc.tensor.matmul(
                                        ps,
                                        wkg[:, ko, :],
                                        xT[:, ko, qc * 512:(qc + 1) * 512]
                                        ,
                                        start=(ko == 0),
                                        stop=(ko == KSUB - 1))
                                nc.vector.tensor_scalar_add(
                                    kraw[:, qc * 512:(qc + 1) * 512], ps, bkt)
                            rope(kraw, KT[:, g, PAST:])

                            # pk head-dim is pre-permuted on the host, so a
                            # plain transpose lands rows in rope layout
                            for kc in range(PAST // 128):
                                pkc = raw.tile([128, HD], f32, tag="pkc")
                                nc.sync.dma_start(
                                    pkc, pk_d[g, kc * 128:(kc + 1) * 128, :])
                                ps = pstp.tile([128, 128], f32, tag="tp")
                                nc.tensor.transpose(ps, pkc, ident)
                                nc.vector.tensor_copy(
                                    KT[:, g, kc * 128:(kc + 1) * 128], ps)
                                nc.sync.dma_start(
                                    V[g][:, kc, :],
                                    pv_d[g, kc * 128:(kc + 1) * 128, :])

                    wk_cm.__exit__(None, None, None)
                    # Q^T (roped), per head
                    with tc.tile_pool(name="wqp", bufs=2) as wqp:
                        for h in range(NH):
                            wqh = wqp.tile([128, KSUB, HD], f32r, tag="wq")
                            nc.sync.dma_start(
                                wqh, wq_d.rearrange("(ko ki) m -> ki ko m",
                                                    ki=128)
                                [:, :, h * HD:(h + 1) * HD])
                            bqt = bias_qk[:, h:h + 1]
                            qraw = raw.tile([128, LQ], f32, tag="raw")
                            for qc in range(QC):
                                ps = psproj.tile([128, 512], f32)
                                for ko in range(KSUB):
                                    nc.tensor.matmul(
                                        ps,
                                        wqh[:, ko, :],
                                        xT[:, ko, qc * 512:(qc + 1) * 512]
                                        ,
                                        start=(ko == 0),
                                        stop=(ko == KSUB - 1))
                                nc.vector.tensor_scalar_add(
                                    qraw[:, qc * 512:(qc + 1) * 512], ps, bqt)
                            rope(qraw, QT[:, h, :])

            # ---- phase 4: attention ----
            import contextlib
            ph45 = contextlib.ExitStack()
            attnp = ph45.enter_context(tc.tile_pool(name="attnp", bufs=1))
            attnT = attnp.tile([128, NH, LQ], f32r)
            wop = ph45.enter_context(tc.tile_pool(name="wop", bufs=2))

            def load_wot(ncH):
                wot = wop.tile([128, NH, 512], f32r, tag="wo",
                               name=f"wo{ncH}")
                nc.sync.dma_start(
                    wot, wo_d.rearrange("(ho hi) n -> hi ho n", hi=128)
                    [:, :, ncH * 512:(ncH + 1) * 512])
                return wot

            wot0 = load_wot(0)
            with (
                tc.tile_pool(name="mpp", bufs=1) as mpp,
                tc.tile_pool(name="ptp", bufs=4) as ptp,
                tc.tile_pool(name="pssc", bufs=3, space="PSUM") as pssc,
                tc.tile_pool(name="pspv", bufs=3, space="PSUM") as pspv,
                tc.tile_pool(name="psdn", bufs=2, space="PSUM") as psdn,
            ):
                mp_sb = None
                if n_part:
                    mp_sb = mpp.tile([128, n_part, 512], adt)
                    for i in range(n_part):
                        nc.sync.dma_start(mp_sb[:, i, :], mp_d[i])

                for h in range(NH):
                    g = h // GS
                    for qc in range(QC):
                        act = active[qc]
                        n_act = len(act)
                        ps_pv = pspv.tile([128, 512], f32)
                        ps_dn = psdn.tile([128, 512], f32)
                        for i, (kc, midx) in enumerate(act):
                            ps_s = pssc.tile([128, 512], f32)
                            nc.tensor.matmul(
                                ps_s,
                                KT[:, g, kc * 128:(kc + 1) * 128]
                                ,
                                QT[:, h, qc * 512:(qc + 1) * 512]
                                ,
                                start=True, stop=True)
                            pt = ptp.tile([128, 512], adt, tag="pt")
                            nc.scalar.activation(pt, ps_s, AF.Exp, scale=scl)
                            if midx is not None:
                                nc.vector.tensor_mul(pt, pt,
                                                     mp_sb[:, midx, :])
                            nc.tensor.matmul(
                                ps_pv, V[g][:, kc, :],
                                pt,
                                start=(i == 0), stop=(i == n_act - 1))
                            nc.tensor.matmul(
                                ps_dn[0:1, :], ones_mat[:, 0:1],
                                pt,
                                start=(i == 0), stop=(i == n_act - 1))
                        rec1 = raw.tile([1, 512], f32, tag="rec1")
                        nc.vector.reciprocal(rec1, ps_dn[0:1, :])
                        rec = raw.tile([128, 512], f32, tag="rec")
                        nc.gpsimd.partition_broadcast(rec, rec1)
                        nc.vector.tensor_mul(
                            attnT[:, h, qc * 512:(qc + 1) * 512], ps_pv, rec)

            # ---- phase 5: output projection ----
            with (
                tc.tile_pool(name="bop", bufs=1) as bop,
                tc.tile_pool(name="pso", bufs=4, space="PSUM") as pso,
            ):
                bo_sb = bop.tile([1, D], f32)
                nc.sync.dma_start(bo_sb, bo_d)
                bo_rep = bop.tile([128, D], f32)
                nc.gpsimd.partition_broadcast(bo_rep, bo_sb)
                for ncH in range(NCH):
                    wot = wot0 if ncH == 0 else load_wot(ncH)
                    for qs in range(QS):
                        ps = pso.tile([128, 512], f32)
                        for h in range(NH):
                            nc.tensor.matmul(
                                ps,
                                attnT[:, h, qs * 128:(qs + 1) * 128]
                                ,
                                wot[:, h, :],
                                start=(h == 0), stop=(h == NH - 1))
                        ot = raw.tile([128, 512], f32, tag="ot")
                        nc.vector.tensor_tensor(
                            ot, ps, bo_rep[:, ncH * 512:(ncH + 1) * 512],
                            OP.add)
                        nc.sync.dma_start(
                            out_d[qs * 128:(qs + 1) * 128,
                                  ncH * 512:(ncH + 1) * 512], ot)
            ph45.close()

    nc.compile()
    return nc


def _classify_mask(mask):
    """Per-[128k x 512q] tile: full / skip / partial (+ fp32 tile data)."""
    mT = mask.T  # [LK, LQ]
    classes = {}
    partials = []
    for qc in range(QC):
        for kc in range(KC):
            t = mT[kc * 128:(kc + 1) * 128, qc * 512:(qc + 1) * 512]
            if t.all():
                classes[(qc, kc)] = ("full", None)
            elif not t.any():
                classes[(qc, kc)] = ("skip", None)
            else:
                classes[(qc, kc)] = ("part", len(partials))
                partials.append(np.ascontiguousarray(t, dtype=np.float32))
    maskp = np.stack(partials) if partials else None
    return classes, maskp


def _prep_in_maps(x, mask, rotary_freqs, past_k, past_v, Wq, bq, Wk, bk,
                  Wv, bv, Wo, bo, maskp, n_part, attn_bf16=False):
    c32 = lambda a: np.ascontiguousarray(a, dtype=np.float32)
    if attn_bf16:
        import ml_dtypes
        cat = lambda a: np.ascontiguousarray(a, dtype=ml_dtypes.bfloat16)
    else:
        cat = c32
    in_maps = []
    for c in range(NCORES):
        b, half = c // 2, c % 2
        h0 = half * NH          # first global head
        g0 = half * NG          # first global group
        wq_c = np.concatenate(
            [Wq[:, (h0 + h) * HD + _PERM] for h in range(NH)], axis=1)
        bq_c = np.stack([bq[(h0 + h) * HD + _PERM] for h in range(NH)])
        wk_c = np.concatenate(
            [Wk[:, (g0 + g) * HD + _PERM] for g in range(NG)], axis=1)
        bk_c = np.stack([bk[(g0 + g) * HD + _PERM] for g in range(NG)])
        m = {
            "x": c32(x[b]),
            "wq": c32(wq_c),
            "bq": c32(bq_c[..., None]),
            "wk": c32(wk_c),
            "bk": c32(bk_c[..., None]),
            "wv": c32(Wv[:, g0 * HD:(g0 + NG) * HD]),
            "bv": c32(bv[g0 * HD:(g0 + NG) * HD][None, :]),
            "pk": c32(past_k[b, g0:g0 + NG][..., _PERM]),
            "pv": cat(past_v[b, g0:g0 + NG]),
            "rot": c32(rotary_freqs),
            "wo": c32(Wo[h0 * HD:(h0 + NH) * HD, :]),
            "bo": c32(bo[None, :] if half == 0 else np.zeros((1, D))),
        }
        if n_part:
            m["maskp"] = cat(maskp)
        in_maps.append(m)
    return in_maps


def _run(inputs, trace=False):
    from concourse import bass_utils

    classes, maskp = _classify_mask(np.asarray(inputs["mask"]))
    n_part = 0 if maskp is None else maskp.shape[0]
    key = (tuple(sorted(classes.items())), ATTN_BF16)
    if key not in _PROG_CACHE:
        _PROG_CACHE[key] = _build_program(classes, n_part,
                                          attn_bf16=ATTN_BF16)
    nc = _PROG_CACHE[key]

    in_maps = _prep_in_maps(
        np.asarray(inputs["x"]), np.asarray(inputs["mask"]),
        np.asarray(inputs["rotary_freqs"]), np.asarray(inputs["past_k"]),
        np.asarray(inputs["past_v"]), np.asarray(inputs["Wq"]),
        np.asarray(inputs["bq"]), np.asarray(inputs["Wk"]),
        np.asarray(inputs["bk"]), np.asarray(inputs["Wv"]),
        np.asarray(inputs["bv"]), np.asarray(inputs["Wo"]),
        np.asarray(inputs["bo"]), maskp, n_part, attn_bf16=ATTN_BF16)

    res = bass_utils.run_bass_kernel_spmd(
        nc, in_maps, list(range(NCORES)), trace=trace,
        trace_cores=list(range(NCORES)) if trace else None)

    out = np.empty((B, LQ, D), np.float32)
    for b in range(B):
        out[b] = res.results[2 * b]["out"] + res.results[2 * b + 1]["out"]
    return out, res


def kernel(**inputs) -> np.ndarray:
    out, _ = _run(inputs, trace=False)
    return out



# revision 11
# speedup vs baseline: 1.4754x; 1.4754x over previous
"""Cached grouped-query multi-head attention on 8 Trainium2 cores.

Sharding: core c -> batch b = c//2, head-half = c%2 (8 of 16 heads, 2 of 4
KV groups per core). Wq/Wk column-parallel, Wo row-parallel; the two
partial Wo products per batch are summed on the host (the "all-reduce").

Device kernel (per core, fp16 data, fp32 PSUM accumulation):
  Host pre-transposes x^T, past_k^T (rope-permuted) and precomputes the
  cos/sin tables, so the device does no transposes at all. All matmuls run
  in fp16 (full PE rate). Attention scores are computed transposed [k, q]
  so softmax probs are PV-ready; exp runs on ScalarE over paired-kc
  [128,1024] PSUM tiles; the softmax denominator is accumulated on the
  vector engine (fp16 adds), partition-reduced on GpSimd, inverted with
  the fast DVE reciprocal, and applied to the PV output. Q projections for
  head h+1 are interleaved into the attention instruction stream of head h
  to keep the PE busy (and at full clock) while ScalarE computes exp.
"""

import math
import sys

import numpy as np

sys.path.insert(0, "/opt/trn_rl_repo")

B, LQ, D = 4, 1024, 2048
H, G = 16, 4
HD = 128            # head dim
GS = H // G         # heads per group
PAST = 1024
LK = PAST + LQ      # 2048
NCORES = 8
NH = 8              # local heads per core
NG = 2              # local groups per core
KSUB = D // 128     # 16 contraction subtiles over D
QC = LQ // 512      # 2 query chunks of 512
QS = LQ // 128      # 8 query subtiles of 128
KC = LK // 128      # 16 key chunks of 128
NCH = D // 512      # 4 output column chunks

_PERM = np.concatenate([np.arange(0, HD, 2), np.arange(1, HD, 2)])
_PROG_CACHE = {}

# how many pending Q-projection work items to drain per attention pair
DRAIN_PER_PAIR = 3


def _build_program(classes, n_part):
    """Build the per-core Bass/Tile program.

    classes[(qc, kc)] = ("full"|"skip"|"part", partial_idx_or_None),
    identical on every core (the mask is shared).
    """
    import concourse.bacc as bacc
    import concourse.mybir as mybir
    import concourse.tile as tile
    from concourse import bass_isa

    f16 = mybir.dt.float16
    f32 = mybir.dt.float32
    AF = mybir.ActivationFunctionType
    OP = mybir.AluOpType

    nc = bacc.Bacc("TRN2", target_bir_lowering=False, debug=False,
                   num_devices=NCORES)

    xt_d = nc.dram_tensor("xt", [128, KSUB, LQ], f16, kind="ExternalInput").ap()
    cosf_d = nc.dram_tensor("cosf", [128, LQ], f16, kind="ExternalInput").ap()
    ssgn_d = nc.dram_tensor("ssgn", [128, LQ], f16, kind="ExternalInput").ap()
    wq_d = nc.dram_tensor("wq", [128, KSUB, NH, HD], f16,
                          kind="ExternalInput").ap()
    bqk_d = nc.dram_tensor("bqk", [128, NH + NG], f32,
                           kind="ExternalInput").ap()
    wk_d = nc.dram_tensor("wk", [128, KSUB, NG, HD], f16,
                          kind="ExternalInput").ap()
    wv_d = nc.dram_tensor("wv", [128, KSUB, NG * HD], f16,
                          kind="ExternalInput").ap()
    bv_d = nc.dram_tensor("bv", [1, NG * HD], f32, kind="ExternalInput").ap()
    pkt_d = nc.dram_tensor("pkt", [128, NG, PAST], f16,
                           kind="ExternalInput").ap()
    pvt_d = nc.dram_tensor("pvt", [128, NG, PAST // 128, HD], f16,
                           kind="ExternalInput").ap()
    wo_d = nc.dram_tensor("wo", [128, NH, D], f16, kind="ExternalInput").ap()
    bo_d = nc.dram_tensor("bo", [1, D], f32, kind="ExternalInput").ap()
    mp_d = None
    if n_part:
        mp_d = nc.dram_tensor("maskp", [128, n_part, 512], f16,
                              kind="ExternalInput").ap()
    out_d = nc.dram_tensor("out", [LQ, D], f32, kind="ExternalOutput").ap()

    # active key chunks per query chunk: list of (kc, partial_idx|None)
    active = {qc: [(kc, classes[(qc, kc)][1])
                   for kc in range(KC) if classes[(qc, kc)][0] != "skip"]
              for qc in range(QC)}

    scl = 1.0 / math.sqrt(HD)

    with tile.TileContext(nc) as tc:
        with (
            tc.tile_pool(name="const", bufs=1) as const,
            tc.tile_pool(name="persist", bufs=1) as persist,
            tc.tile_pool(name="rawp", bufs=2) as rawp,
            tc.tile_pool(name="ropep", bufs=2) as ropep,
        ):
            xt = persist.tile([128, KSUB, LQ], f16)
            QT = persist.tile([128, NH, LQ], f16)       # roped Q^T (perm rows)
            KT = persist.tile([128, NG, LK], f16)       # K^T cache (perm rows)
            V = persist.tile([128, NG, KC, HD], f16)
            attnT = persist.tile([128, NH, LQ], f16)
            # full-height rotary tables: both halves hold the 64 freqs;
            # ssgn carries -sin on top, +sin on bottom, so
            #   roped = src*cosf + swap(src)*ssgn
            cosf = const.tile([128, LQ], f16)
            ssgn = const.tile([128, LQ], f16)
            bqk = const.tile([128, NH + NG], f32)
            bv_sb = const.tile([1, NG * HD], f32)
            bv_rep = const.tile([128, NG * HD], f32)
            bo_sb = const.tile([1, D], f32)
            bo_rep = const.tile([128, D], f32)
            mp_sb = None

            # ---- input DMAs (queue order = start order) ----
            for qc in range(QC):
                nc.sync.dma_start(xt[:, :, qc * 512:(qc + 1) * 512],
                                  xt_d[:, :, qc * 512:(qc + 1) * 512])
            nc.sync.dma_start(cosf, cosf_d)
            nc.sync.dma_start(ssgn, ssgn_d)
            nc.sync.dma_start(bqk, bqk_d)
            nc.sync.dma_start(KT[:, :, 0:PAST], pkt_d)
            nc.sync.dma_start(V[:, :, 0:PAST // 128, :], pvt_d)
            nc.sync.dma_start(bv_sb, bv_d)
            nc.sync.dma_start(bo_sb, bo_d)
            if n_part:
                mp_sb = const.tile([128, n_part, 512], f16)
                nc.sync.dma_start(mp_sb, mp_d)
            nc.gpsimd.partition_broadcast(bv_rep, bv_sb)
            nc.gpsimd.partition_broadcast(bo_rep, bo_sb)

            def rope(src, dst):
                # src/dst [128, LQ] f16; rows 0:64 = even dims, 64:128 = odd
                # swap exchanges the partition halves (x1<->x2):
                #   top: x1*cos + x2*(-sin)   bot: x2*cos + x1*(+sin)
                swp = ropep.tile([128, LQ], f16, tag="swp")
                nc.sync.dma_start(swp[0:64], src[64:128])
                nc.sync.dma_start(swp[64:128], src[0:64])
                t = ropep.tile([128, LQ], f16, tag="ropet")
                nc.vector.tensor_mul(t, swp, ssgn)
                nc.vector.tensor_mul(dst, src, cosf)
                nc.vector.tensor_tensor(dst, dst, t, OP.add)

            # ---- phase A: K and V projections (+ KV cache loads above) ----
            wop_cm = tc.tile_pool(name="wop", bufs=2)
            wop = wop_cm.__enter__()

            wkv_cm = tc.tile_pool(name="wkvp", bufs=1)
            wkvp = wkv_cm.__enter__()
            wk_sb = wkvp.tile([128, KSUB, NG, HD], f16)
            wv_sb = wkvp.tile([128, KSUB, NG * HD], f16)
            nc.sync.dma_start(wk_sb, wk_d)
            nc.sync.dma_start(wv_sb, wv_d)

            wqp_cm = tc.tile_pool(name="wqp", bufs=2)
            wqp = wqp_cm.__enter__()

            def load_wq(h):
                wqh = wqp.tile([128, KSUB, HD], f16, tag="wq", name=f"wq{h}")
                nc.sync.dma_start(wqh, wq_d[:, :, h, :])
                return wqh

            wq0 = load_wq(0)

            def load_wot(ncH):
                wot = wop.tile([128, NH, 512], f16, tag="wo", name=f"wo{ncH}")
                nc.sync.dma_start(wot, wo_d[:, :, ncH * 512:(ncH + 1) * 512])
                return wot

            with tc.tile_pool(name="psA", bufs=2, space="PSUM") as psA:
                kraws = [rawp.tile([128, LQ], f16, tag=f"kraw{g}",
                                   name=f"kraw{g}") for g in range(NG)]
                for qc in range(QC):
                    for g in range(NG):
                        ps = psA.tile([128, 512], f32, tag="k")
                        for ko in range(KSUB):
                            nc.tensor.matmul(
                                ps, wk_sb[:, ko, g, :],
                                xt[:, ko, qc * 512:(qc + 1) * 512],
                                start=(ko == 0), stop=(ko == KSUB - 1))
                        nc.vector.tensor_scalar_add(
                            kraws[g][:, qc * 512:(qc + 1) * 512], ps,
                            bqk[:, NH + g:NH + g + 1])
                for g in range(NG):
                    rope(kraws[g], KT[:, g, PAST:])

                for qs in range(QS):
                    ps = psA.tile([128, NG * HD], f32, tag="v")
                    for ko in range(KSUB):
                        nc.tensor.matmul(
                            ps, xt[:, ko, qs * 128:(qs + 1) * 128],
                            wv_sb[:, ko, :],
                            start=(ko == 0), stop=(ko == KSUB - 1))
                    nc.vector.tensor_tensor(
                        V[:, :, PAST // 128 + qs, :],
                        ps.rearrange("p (g m) -> p g m", g=NG),
                        bv_rep.rearrange("p (g m) -> p g m", g=NG), OP.add)

            wot0 = load_wot(0)

            # ---- phases B+C: Q projections interleaved with attention ----
            with (
                tc.tile_pool(name="pssc", bufs=2, space="PSUM") as pssc,
                tc.tile_pool(name="pspv", bufs=2, space="PSUM") as pspv,
                tc.tile_pool(name="psq", bufs=2, space="PSUM") as psq,
                tc.tile_pool(name="ptp", bufs=3) as ptp,
                tc.tile_pool(name="denp", bufs=2) as denp,
                tc.tile_pool(name="nrm", bufs=2) as nrm,
            ):
                def make_qproj_items(h, wqh):
                    """Work items building QT[h]: 32 MMs + 2 bias + rope."""
                    items = []
                    qraw = rawp.tile([128, LQ], f16, tag="qraw", name="qraw")
                    pss = [psq.tile([128, 512], f32, tag="q", name=f"q{qc}")
                           for qc in range(QC)]

                    def mm(qc, ko):
                        return lambda: nc.tensor.matmul(
                            pss[qc], wqh[:, ko, :],
                            xt[:, ko, qc * 512:(qc + 1) * 512],
                            start=(ko == 0), stop=(ko == KSUB - 1))

                    def bias(qc):
                        return lambda: nc.vector.tensor_scalar_add(
                            qraw[:, qc * 512:(qc + 1) * 512], pss[qc],
                            bqk[:, h:h + 1])

                    for qc in range(QC):
                        for ko in range(KSUB):
                            items.append(mm(qc, ko))
                        items.append(bias(qc))
                    items.append(lambda: rope(qraw, QT[:, h, :]))
                    return items

                pending = []

                def drain(k):
                    for _ in range(min(k, len(pending))):
                        pending.pop(0)()

                for it in make_qproj_items(0, wq0):
                    it()

                for h in range(NH):
                    g = h // GS
                    if h + 1 < NH:
                        wqh = load_wq(h + 1)
                        pending.extend(make_qproj_items(h + 1, wqh))
                    for qc in range(QC):
                        act = active[qc]
                        n_act = len(act)
                        ps_pv = pspv.tile([128, 512], f32)
                        den = denp.tile([128, 512], f16, tag="den")
                        idx = 0
                        for p0 in range(0, n_act, 2):
                            pair = act[p0:p0 + 2]
                            pss = pssc.tile([128, 1024], f32)
                            for j, (kc, midx) in enumerate(pair):
                                nc.tensor.matmul(
                                    pss[:, j * 512:(j + 1) * 512],
                                    KT[:, g, kc * 128:(kc + 1) * 128],
                                    QT[:, h, qc * 512:(qc + 1) * 512],
                                    start=True, stop=True)
                            drain(DRAIN_PER_PAIR)
                            pt = ptp.tile([128, 1024], f16, tag="pt")
                            w = len(pair) * 512
                            nc.scalar.activation(pt[:, 0:w], pss[:, 0:w],
                                                 AF.Exp, scale=scl)
                            for j, (kc, midx) in enumerate(pair):
                                if midx is not None:
                                    nc.vector.tensor_mul(
                                        pt[:, j * 512:(j + 1) * 512],
                                        pt[:, j * 512:(j + 1) * 512],
                                        mp_sb[:, midx, :])
                            if p0 == 0 and len(pair) == 2:
                                nc.vector.tensor_tensor(
                                    den, pt[:, 0:512], pt[:, 512:1024], OP.add)
                            else:
                                for j in range(len(pair)):
                                    nc.vector.tensor_tensor(
                                        den, den,
                                        pt[:, j * 512:(j + 1) * 512], OP.add)
                            for j, (kc, midx) in enumerate(pair):
                                nc.tensor.matmul(
                                    ps_pv, V[:, g, kc, :],
                                    pt[:, j * 512:(j + 1) * 512],
                                    start=(idx == 0), stop=(idx == n_act - 1))
                                idx += 1
                        denf = nrm.tile([128, 512], f32, tag="denf")
                        nc.gpsimd.partition_all_reduce(
                            denf, den, 128, bass_isa.ReduceOp.add)
                        rec = nrm.tile([128, 512], f32, tag="rec")
                        nc.vector.reciprocal_approx_fast(rec, denf)
                        nc.vector.tensor_mul(
                            attnT[:, h, qc * 512:(qc + 1) * 512], ps_pv, rec)
                    drain(len(pending))

            wqp_cm.__exit__(None, None, None)
            wkv_cm.__exit__(None, None, None)

            # ---- phase D: output projection ----
            with (
                tc.tile_pool(name="pso", bufs=4, space="PSUM") as pso,
                tc.tile_pool(name="outp", bufs=3) as outp,
            ):
                for ncH in range(NCH):
                    wot = wot0 if ncH == 0 else load_wot(ncH)
                    if ncH + 1 < NCH and ncH == 0:
                        pass
                    for qs in range(QS):
                        ps = pso.tile([128, 512], f32)
                        for hh in range(NH):
                            nc.tensor.matmul(
                                ps, attnT[:, hh, qs * 128:(qs + 1) * 128],
                                wot[:, hh, :],
                                start=(hh == 0), stop=(hh == NH - 1))
                        ot = outp.tile([128, 512], f32, tag="ot")
                        nc.vector.tensor_tensor(
                            ot, ps, bo_rep[:, ncH * 512:(ncH + 1) * 512],
                            OP.add)
                        nc.sync.dma_start(
                            out_d[qs * 128:(qs + 1) * 128,
                                  ncH * 512:(ncH + 1) * 512], ot)
            wop_cm.__exit__(None, None, None)

    nc.compile()
    return nc


def _classify_mask(mask):
    """Per-[128k x 512q] tile: full / skip / partial (+ fp16 tile data)."""
    mT = mask.T  # [LK, LQ]
    classes = {}
    partials = []
    for qc in range(QC):
        for kc in range(KC):
            t = mT[kc * 128:(kc + 1) * 128, qc * 512:(qc + 1) * 512]
            if t.all():
                classes[(qc, kc)] = ("full", None)
            elif not t.any():
                classes[(qc, kc)] = ("skip", None)
            else:
                classes[(qc, kc)] = ("part", len(partials))
                partials.append(np.ascontiguousarray(t, dtype=np.float16))
    # [128, n_part, 512] layout for a single DMA
    maskp = (np.ascontiguousarray(np.stack(partials).transpose(1, 0, 2))
             if partials else None)
    return classes, maskp


def _prep_in_maps(x, mask, rotary_freqs, past_k, past_v, Wq, bq, Wk, bk,
                  Wv, bv, Wo, bo, maskp, n_part):
    c32 = lambda a: np.ascontiguousarray(a, dtype=np.float32)
    c16 = lambda a: np.ascontiguousarray(a, dtype=np.float16)
    cosT = np.cos(rotary_freqs).T       # [64, LQ]
    sinT = np.sin(rotary_freqs).T
    cosf = c16(np.concatenate([cosT, cosT], axis=0))    # [128, LQ]
    ssgn = c16(np.concatenate([-sinT, sinT], axis=0))   # [128, LQ]
    in_maps = []
    for c in range(NCORES):
        b, half = c // 2, c % 2
        h0 = half * NH          # first global head
        g0 = half * NG          # first global group
        qcols = np.concatenate(
            [(h0 + h) * HD + _PERM for h in range(NH)])
        kcols = np.concatenate(
            [(g0 + g) * HD + _PERM for g in range(NG)])
        wq_c = Wq[:, qcols].reshape(KSUB, 128, NH, HD).transpose(1, 0, 2, 3)
        wk_c = Wk[:, kcols].reshape(KSUB, 128, NG, HD).transpose(1, 0, 2, 3)
        wv_c = (Wv[:, g0 * HD:(g0 + NG) * HD]
                .reshape(KSUB, 128, NG * HD).transpose(1, 0, 2))
        bqk = np.stack(
            [bq[(h0 + h) * HD + _PERM] for h in range(NH)]
            + [bk[(g0 + g) * HD + _PERM] for g in range(NG)], axis=1)
        pkt = np.stack(
            [past_k[b, g0 + g][:, _PERM].T for g in range(NG)], axis=1)
        pvt = np.stack(
            [past_v[b, g0 + g].reshape(PAST // 128, 128, HD).transpose(1, 0, 2)
             for g in range(NG)], axis=1)
        wo_c = (Wo[h0 * HD:(h0 + NH) * HD, :]
                .reshape(NH, 128, D).transpose(1, 0, 2))
        xt = x[b].T.reshape(KSUB, 128, LQ).transpose(1, 0, 2)
        m = {
            "xt": c16(xt),
            "cosf": cosf,
            "ssgn": ssgn,
            "wq": c16(wq_c),
            "bqk": c32(bqk),
            "wk": c16(wk_c),
            "wv": c16(wv_c),
            "bv": c32(bv[g0 * HD:(g0 + NG) * HD][None, :]),
            "pkt": c16(pkt),
            "pvt": c16(pvt),
            "wo": c16(wo_c),
            "bo": c32(bo[None, :] if half == 0 else np.zeros((1, D))),
        }
        if n_part:
            m["maskp"] = maskp
        in_maps.append(m)
    return in_maps


def _run(inputs, trace=False):
    from concourse import bass_utils

    classes, maskp = _classify_mask(np.asarray(inputs["mask"]))
    n_part = 0 if maskp is None else maskp.shape[1]
    key = tuple(sorted(classes.items()))
    if key not in _PROG_CACHE:
        _PROG_CACHE[key] = _build_program(classes, n_part)
    nc = _PROG_CACHE[key]

    in_maps = _prep_in_maps(
        np.asarray(inputs["x"]), np.asarray(inputs["mask"]),
        np.asarray(inputs["rotary_freqs"]), np.asarray(inputs["past_k"]),
        np.asarray(inputs["past_v"]), np.asarray(inputs["Wq"]),
        np.asarray(inputs["bq"]), np.asarray(inputs["Wk"]),
        np.asarray(inputs["bk"]), np.asarray(inputs["Wv"]),
        np.asarray(inputs["bv"]), np.asarray(inputs["Wo"]),
        np.asarray(inputs["bo"]), maskp, n_part)

    res = bass_utils.run_bass_kernel_spmd(
        nc, in_maps, list(range(NCORES)), trace=trace,
        trace_cores=list(range(NCORES)) if trace else None)

    out = np.empty((B, LQ, D), np.float32)
    for b in range(B):
        out[b] = res.results[2 * b]["out"] + res.results[2 * b + 1]["out"]
    return out, res


def kernel(**inputs) -> np.ndarray:
    out, _ = _run(inputs, trace=False)
    return out


# revision 22
# speedup vs baseline: 1.5496x; 1.0503x over previous
"""Cached grouped-query multi-head attention on 8 Trainium2 cores.

Sharding: core c -> batch b = c//2, head-half = c%2 (8 of 16 heads, 2 of 4
KV groups per core). Wq/Wk column-parallel, Wo row-parallel; the two
partial Wo products per batch are summed on the host (the "all-reduce").

Device kernel (per core, fp16 data, fp32 PSUM accumulation):
  Host pre-transposes x^T, past_k^T (rope-permuted) and precomputes the
  cos/sin tables, so the device does no transposes. All matmuls are fp16.
  Attention is kc-major: scores for both 512-query chunks of one key
  chunk land in one [128,1024] PSUM tile (the KT/V stationary is loaded
  once and reused via ldweights=False on the second matmul), one exp per
  key chunk covers both, and the two PV accumulations run in the halves
  of a single [128,1024] PSUM tile. The softmax denominator accumulates
  on the vector engine (one fp16 add per key chunk), is partition-reduced
  once per head on GpSimd, inverted with the fast DVE reciprocal, and
  applied to the copied-out PV result. Q projections for head h+1 are
  interleaved into head h's attention stream to keep the PE busy while
  ScalarE computes exp. The output projection shares each attnT
  stationary across the four output-column chunks.
"""

import math
import sys

import numpy as np

sys.path.insert(0, "/opt/trn_rl_repo")

B, LQ, D = 4, 1024, 2048
H, G = 16, 4
HD = 128            # head dim
GS = H // G         # heads per group
PAST = 1024
LK = PAST + LQ      # 2048
NCORES = 8
NH = 8              # local heads per core
NG = 2              # local groups per core
KSUB = D // 128     # 16 contraction subtiles over D
QC = LQ // 512      # 2 query chunks of 512
QS = LQ // 128      # 8 query subtiles of 128
KC = LK // 128      # 16 key chunks of 128
NCH = D // 512      # 4 output column chunks

_PERM = np.concatenate([np.arange(0, HD, 2), np.arange(1, HD, 2)])
_PROG_CACHE = {}

DRAIN_PER_KC = 3    # pending Q-proj work items drained per attention kc step


def _build_program(classes, n_part):
    """Build the per-core Bass/Tile program.

    classes[(qc, kc)] = ("full", None, None) | ("skip", None, None) |
    ("part", partial_idx, mask_width); identical on every core.
    """
    import concourse.bacc as bacc
    import concourse.mybir as mybir
    import concourse.tile as tile
    from concourse import bass_isa

    f16 = mybir.dt.float16
    f32 = mybir.dt.float32
    AF = mybir.ActivationFunctionType
    OP = mybir.AluOpType

    nc = bacc.Bacc("TRN2", target_bir_lowering=False, debug=False,
                   num_devices=NCORES)

    xt_d = nc.dram_tensor("xt", [128, KSUB, LQ], f16, kind="ExternalInput").ap()
    cosf_d = nc.dram_tensor("cosf", [128, LQ], f16, kind="ExternalInput").ap()
    ssgn_d = nc.dram_tensor("ssgn", [128, LQ], f16, kind="ExternalInput").ap()
    wq_d = nc.dram_tensor("wq", [128, KSUB, NH, HD], f16,
                          kind="ExternalInput").ap()
    bqk_d = nc.dram_tensor("bqk", [128, NH + NG], f32,
                           kind="ExternalInput").ap()
    wk_d = nc.dram_tensor("wk", [128, KSUB, NG, HD], f16,
                          kind="ExternalInput").ap()
    wv_d = nc.dram_tensor("wv", [128, KSUB, NG * HD], f16,
                          kind="ExternalInput").ap()
    bv_d = nc.dram_tensor("bv", [1, NG * HD], f32, kind="ExternalInput").ap()
    pkt_d = nc.dram_tensor("pkt", [128, NG, PAST], f16,
                           kind="ExternalInput").ap()
    pvt_d = nc.dram_tensor("pvt", [128, NG, PAST // 128, HD], f16,
                           kind="ExternalInput").ap()
    wo_d = nc.dram_tensor("wo", [128, NH, D], f16, kind="ExternalInput").ap()
    bo_d = nc.dram_tensor("bo", [1, D], f32, kind="ExternalInput").ap()
    mp_d = None
    if n_part:
        mp_d = nc.dram_tensor("maskp", [128, n_part, 512], f16,
                              kind="ExternalInput").ap()
    out_d = nc.dram_tensor("out", [LQ, D], f32, kind="ExternalOutput").ap()

    def cls(qc, kc):
        return classes[(qc, kc)]

    active = {qc: [kc for kc in range(KC) if cls(qc, kc)[0] != "skip"]
              for qc in range(QC)}
    S = [kc for kc in active[1] if kc in active[0]]     # both chunks active
    E1 = [kc for kc in active[1] if kc not in active[0]]  # q-chunk-1 only
    E0 = [kc for kc in active[0] if kc not in active[1]]  # q-chunk-0 only
    assert not E0, "unexpected mask shape (q-chunk-0-only key chunk)"

    scl = 1.0 / math.sqrt(HD)

    with tile.TileContext(nc) as tc:
        with (
            tc.tile_pool(name="const", bufs=1) as const,
            tc.tile_pool(name="persist", bufs=1) as persist,
            tc.tile_pool(name="rawp", bufs=2) as rawp,
            tc.tile_pool(name="ropep", bufs=2) as ropep,
        ):
            xt = persist.tile([128, KSUB, LQ], f16)
            QT = persist.tile([128, NH, LQ], f16)       # roped Q^T (perm rows)
            KT = persist.tile([128, NG, LK], f16)       # K^T cache (perm rows)
            V = persist.tile([128, NG, KC, HD], f16)
            attnT = persist.tile([128, NH, LQ], f16)
            wo_sb = persist.tile([128, NH, D], f16)
            cosf = const.tile([128, LQ], f16)
            ssgn = const.tile([128, LQ], f16)
            bqk = const.tile([128, NH + NG], f32)
            bv_sb = const.tile([1, NG * HD], f32)
            bv_rep = const.tile([128, NG * HD], f32)
            bo_sb = const.tile([1, D], f32)
            bo_rep = const.tile([128, D], f32)
            mp_sb = None

            wqp_cm = tc.tile_pool(name="wqp", bufs=2)
            wqp = wqp_cm.__enter__()

            def load_wq(h):
                wqh = wqp.tile([128, KSUB, HD], f16, tag="wq", name=f"wq{h}")
                nc.sync.dma_start(wqh, wq_d[:, :, h, :])
                return wqh

            # ---- input DMAs (queue order = start order) ----
            nc.sync.dma_start(xt[:, :, 0:512], xt_d[:, :, 0:512])
            wq0 = load_wq(0)
            nc.sync.dma_start(bqk, bqk_d)
            wkv_cm = tc.tile_pool(name="wkvp", bufs=1)
            wkvp = wkv_cm.__enter__()
            wk_sb = wkvp.tile([128, KSUB, NG, HD], f16)
            wv_sb = wkvp.tile([128, KSUB, NG * HD], f16)
            nc.sync.dma_start(wk_sb, wk_d)
            nc.sync.dma_start(xt[:, :, 512:1024], xt_d[:, :, 512:1024])
            nc.sync.dma_start(cosf, cosf_d)
            nc.sync.dma_start(ssgn, ssgn_d)
            nc.sync.dma_start(wv_sb, wv_d)
            nc.sync.dma_start(bv_sb, bv_d)
            nc.sync.dma_start(KT[:, :, 0:PAST], pkt_d)
            nc.sync.dma_start(V[:, :, 0:PAST // 128, :], pvt_d)
            nc.sync.dma_start(bo_sb, bo_d)
            if n_part:
                mp_sb = const.tile([128, n_part, 512], f16)
                nc.sync.dma_start(mp_sb, mp_d)
            nc.sync.dma_start(wo_sb, wo_d)
            nc.gpsimd.partition_broadcast(bv_rep, bv_sb)
            nc.gpsimd.partition_broadcast(bo_rep, bo_sb)

            def rope_swaps(src, swp, qc):
                s, e = qc * 512, (qc + 1) * 512
                nc.sync.dma_start(swp[0:64, s:e], src[64:128, s:e])
                nc.sync.dma_start(swp[64:128, s:e], src[0:64, s:e])

            def rope_muls(src, swp, dst):
                t = ropep.tile([128, LQ], f16, tag="ropet", name="ropet")
                nc.vector.tensor_mul(t, swp, ssgn)
                nc.vector.tensor_mul(dst, src, cosf)
                nc.vector.tensor_tensor(dst, dst, t, OP.add)

            def sbias(out, ps, col):
                nc.scalar.activation(out, ps, AF.Identity,
                                     bias=bqk[:, col:col + 1])

            # ---- phase A: Q0/K/V projections ----
            with tc.tile_pool(name="psA", bufs=2, space="PSUM") as psA:
                # Q proj head 0, q-chunk 0 (earliest DMAs), then K qc0
                qraw0 = rawp.tile([128, LQ], f16, tag="qraw", name="qraw0")
                psq0 = [None, None]
                for qc in range(QC):
                    psq0[qc] = psA.tile([128, 512], f32, tag="q",
                                        name=f"q{qc}")
                kraws = [rawp.tile([128, LQ], f16, tag=f"kraw{g}",
                                   name=f"kraw{g}", bufs=1) for g in range(NG)]
                kswp = [ropep.tile([128, LQ], f16, tag=f"kswp{g}",
                                   name=f"kswp{g}", bufs=1) for g in range(NG)]
                qswp = ropep.tile([128, LQ], f16, tag="qswp", name="qswp0")
                psk = {}
                for qc in range(QC):
                    for ko in range(KSUB):
                        nc.tensor.matmul(
                            psq0[qc], wq0[:, ko, :],
                            xt[:, ko, qc * 512:(qc + 1) * 512],
                            start=(ko == 0), stop=(ko == KSUB - 1))
                    sbias(qraw0[:, qc * 512:(qc + 1) * 512], psq0[qc], 0)
                    rope_swaps(qraw0, qswp, qc)
                    for g in range(NG):
                        ps = psA.tile([128, 512], f32, tag="k",
                                      name=f"k{g}{qc}")
                        for ko in range(KSUB):
                            nc.tensor.matmul(
                                ps, wk_sb[:, ko, g, :],
                                xt[:, ko, qc * 512:(qc + 1) * 512],
                                start=(ko == 0), stop=(ko == KSUB - 1))
                        sbias(kraws[g][:, qc * 512:(qc + 1) * 512], ps,
                              NH + g)
                        rope_swaps(kraws[g], kswp[g], qc)
                rope_muls(qraw0, qswp, QT[:, 0, :])
                for g in range(NG):
                    rope_muls(kraws[g], kswp[g], KT[:, g, PAST:])

                for qs in range(QS):
                    ps = psA.tile([128, NG * HD], f32, tag="v")
                    for ko in range(KSUB):
                        nc.tensor.matmul(
                            ps, xt[:, ko, qs * 128:(qs + 1) * 128],
                            wv_sb[:, ko, :],
                            start=(ko == 0), stop=(ko == KSUB - 1))
                    nc.vector.tensor_tensor(
                        V[:, :, PAST // 128 + qs, :],
                        ps.rearrange("p (g m) -> p g m", g=NG),
                        bv_rep.rearrange("p (g m) -> p g m", g=NG), OP.add)

            # ---- phases B+C: Q projections interleaved with attention ----
            with (
                tc.tile_pool(name="pssc", bufs=2, space="PSUM") as pssc,
                tc.tile_pool(name="pspv", bufs=1, space="PSUM") as pspv,
                tc.tile_pool(name="psq", bufs=2, space="PSUM") as psq,
                tc.tile_pool(name="ptp", bufs=3) as ptp,
                tc.tile_pool(name="denp", bufs=2) as denp,
                tc.tile_pool(name="nrm", bufs=1) as nrm,
                tc.tile_pool(name="pvf", bufs=1) as pvf,
            ):
                def make_qproj_items(h, wqh):
                    """Work items building QT[h]: 16 MM-pairs + bias + rope.

                    ko-outer so each weight subtile is loaded once and
                    reused (ldweights=False) for the second query chunk.
                    """
                    items = []
                    qraw = rawp.tile([128, LQ], f16, tag="qraw", name="qraw")
                    swp = ropep.tile([128, LQ], f16, tag="qswp", name="qswp")
                    pss = [psq.tile([128, 512], f32, tag="q", name=f"q{qc}")
                           for qc in range(QC)]

                    def mmpair(ko):
                        def go():
                            nc.tensor.matmul(
                                pss[0], wqh[:, ko, :], xt[:, ko, 0:512],
                                start=(ko == 0), stop=(ko == KSUB - 1),
                                skip_group_check=True)
                            m1 = nc.tensor.matmul(
                                pss[1], wqh[:, ko, :], xt[:, ko, 512:1024],
                                start=(ko == 0), stop=(ko == KSUB - 1),
                                skip_group_check=True)
                            m1.ins.ldweights = False
                        return go

                    def bias_swap(qc):
                        def go():
                            sbias(qraw[:, qc * 512:(qc + 1) * 512], pss[qc],
                                  h)
                            rope_swaps(qraw, swp, qc)
                        return go

                    for ko in range(KSUB):
                        items.append(mmpair(ko))
                    items.append(bias_swap(0))
                    items.append(bias_swap(1))
                    items.append(lambda: rope_muls(qraw, swp, QT[:, h, :]))
                    return items

                pending = []

                def drain(k):
                    for _ in range(min(k, len(pending))):
                        pending.pop(0)()

                for h in range(NH):
                    g = h // GS
                    if h + 1 < NH:
                        wqh = load_wq(h + 1)
                        pending.extend(make_qproj_items(h + 1, wqh))
                    ps_pv = pspv.tile([128, 1024], f32)
                    den = denp.tile([128, 1024], f16, tag="den")
                    pvt_h = pvf.tile([128, 1024], f16, tag="pvt")

                    def attn_step(kc_qcs):
                        # kc_qcs: list of (kc, qc) with shared pss tile;
                        # single kc with both qcs shares the KT stationary.
                        pss = pssc.tile([128, 1024], f32, tag="sc",
                                        name="sc")
                        shared = (len(kc_qcs) == 2
                                  and kc_qcs[0][0] == kc_qcs[1][0])
                        for j, (kc, qc) in enumerate(kc_qcs):
                            m = nc.tensor.matmul(
                                pss[:, j * 512:(j + 1) * 512],
                                KT[:, g, kc * 128:(kc + 1) * 128],
                                QT[:, h, qc * 512:(qc + 1) * 512],
                                start=True, stop=True,
                                skip_group_check=True)
                            if shared and j == 1:
                                m.ins.ldweights = False
                        drain(DRAIN_PER_KC)
                        w = len(kc_qcs) * 512
                        pt = ptp.tile([128, 1024], f16, tag="pt", name="pt")
                        nc.scalar.activation(pt[:, 0:w], pss[:, 0:w],
                                             AF.Exp, scale=scl)
                        for j, (kc, qc) in enumerate(kc_qcs):
                            kind, midx, mw = cls(qc, kc)
                            if kind == "part":
                                nc.vector.tensor_mul(
                                    pt[:, j * 512:j * 512 + mw],
                                    pt[:, j * 512:j * 512 + mw],
                                    mp_sb[:, midx, 0:mw])
                        if shared:
                            nc.vector.tensor_tensor(den, den, pt, OP.add)
                        else:
                            for j, (kc, qc) in enumerate(kc_qcs):
                                nc.vector.tensor_tensor(
                                    den[:, qc * 512:(qc + 1) * 512],
                                    den[:, qc * 512:(qc + 1) * 512],
                                    pt[:, j * 512:(j + 1) * 512], OP.add)
                        for j, (kc, qc) in enumerate(kc_qcs):
                            mm = nc.tensor.matmul(
                                ps_pv[:, qc * 512:(qc + 1) * 512],
                                V[:, g, kc, :],
                                pt[:, j * 512:(j + 1) * 512],
                                start=(kc == S[0]),
                                stop=(kc == (S[-1] if qc == 0 else
                                             (E1[-1] if E1 else S[-1]))),
                                skip_group_check=True)
                            if shared and j == 1:
                                mm.ins.ldweights = False

                    # den starts at zero: first step writes, rest accumulate
                    first = S[0]
                    pss0 = pssc.tile([128, 1024], f32, tag="sc", name="sc0")
                    m = nc.tensor.matmul(
                        pss0[:, 0:512], KT[:, g, first * 128:first * 128 + 128],
                        QT[:, h, 0:512], start=True, stop=True,
                        skip_group_check=True)
                    m1 = nc.tensor.matmul(
                        pss0[:, 512:1024],
                        KT[:, g, first * 128:first * 128 + 128],
                        QT[:, h, 512:1024], start=True, stop=True,
                        skip_group_check=True)
                    m1.ins.ldweights = False
                    drain(DRAIN_PER_KC)
                    pt0 = ptp.tile([128, 1024], f16, tag="pt", name="pt0")
                    nc.scalar.activation(pt0, pss0, AF.Exp, scale=scl)
                    for qc in range(QC):
                        kind, midx, mw = cls(qc, first)
                        if kind == "part":
                            nc.vector.tensor_mul(
                                pt0[:, qc * 512:qc * 512 + mw],
                                pt0[:, qc * 512:qc * 512 + mw],
                                mp_sb[:, midx, 0:mw])
                    nc.vector.tensor_copy(den, pt0)
                    for qc in range(QC):
                        mm = nc.tensor.matmul(
                            ps_pv[:, qc * 512:(qc + 1) * 512],
                            V[:, g, first, :],
                            pt0[:, qc * 512:(qc + 1) * 512],
                            start=True, stop=False, skip_group_check=True)
                        if qc == 1:
                            mm.ins.ldweights = False

                    for kc in S[1:]:
                        attn_step([(kc, 0), (kc, 1)])
                    nc.vector.tensor_copy(pvt_h[:, 0:512], ps_pv[:, 0:512])
                    for i in range(0, len(E1), 2):
                        attn_step([(kc, 1) for kc in E1[i:i + 2]])
                    nc.vector.tensor_copy(pvt_h[:, 512:1024],
                                          ps_pv[:, 512:1024])

                    for qc in range(QC):
                        s, e = qc * 512, (qc + 1) * 512
                        denf = nrm.tile([128, 512], f32, tag="denf",
                                        name="denf", bufs=1)
                        nc.gpsimd.partition_all_reduce(
                            denf, den[:, s:e], 128, bass_isa.ReduceOp.add)
                        rec = nrm.tile([128, 512], f32, tag="rec",
                                       name="rec", bufs=1)
                        nc.vector.reciprocal_approx_fast(rec, denf)
                        nc.vector.tensor_mul(attnT[:, h, s:e],
                                             pvt_h[:, s:e], rec)
                    drain(len(pending))

            wkv_cm.__exit__(None, None, None)
            wqp_cm.__exit__(None, None, None)

            # ---- phase D: output projection ----
            # qs-outer with all 4 output-column accumulators live so each
            # attnT stationary is loaded once and reused 4x.
            with (
                tc.tile_pool(name="pso", bufs=8, space="PSUM") as pso,
                tc.tile_pool(name="outp", bufs=4) as outp,
            ):
                for qs in range(QS):
                    pss = [pso.tile([128, 512], f32, tag="o", name=f"o{i}")
                           for i in range(NCH)]
                    for hh in range(NH):
                        for ncH in range(NCH):
                            m = nc.tensor.matmul(
                                pss[ncH],
                                attnT[:, hh, qs * 128:(qs + 1) * 128],
                                wo_sb[:, hh, ncH * 512:(ncH + 1) * 512],
                                start=(hh == 0), stop=(hh == NH - 1),
                                skip_group_check=True)
                            if ncH:
                                m.ins.ldweights = False
                    for ncH in range(NCH):
                        ot = outp.tile([128, 512], f32, tag="ot", name="ot")
                        nc.vector.tensor_tensor(
                            ot, pss[ncH],
                            bo_rep[:, ncH * 512:(ncH + 1) * 512], OP.add)
                        nc.sync.dma_start(
                            out_d[qs * 128:(qs + 1) * 128,
                                  ncH * 512:(ncH + 1) * 512], ot)

    nc.compile()
    return nc


def _classify_mask(mask):
    """Per-[128k x 512q] tile: full / skip / partial (+ fp16 tile data).

    Partial tiles also record the mask width: the column count after which
    every row of the tile is valid (the multiply is sliced to that width).
    """
    mT = mask.T  # [LK, LQ]
    classes = {}
    partials = []
    for qc in range(QC):
        for kc in range(KC):
            t = mT[kc * 128:(kc + 1) * 128, qc * 512:(qc + 1) * 512]
            if t.all():
                classes[(qc, kc)] = ("full", None, None)
            elif not t.any():
                classes[(qc, kc)] = ("skip", None, None)
            else:
                colfull = t.all(axis=0)
                nz = np.where(~colfull)[0]
                w = int(nz.max()) + 1 if len(nz) else 0
                w = min(512, (w + 3) // 4 * 4)
                classes[(qc, kc)] = ("part", len(partials), w)
                partials.append(np.ascontiguousarray(t, dtype=np.float16))
    # [128, n_part, 512] layout for a single DMA
    maskp = (np.ascontiguousarray(np.stack(partials).transpose(1, 0, 2))
             if partials else None)
    return classes, maskp


def _prep_in_maps(x, mask, rotary_freqs, past_k, past_v, Wq, bq, Wk, bk,
                  Wv, bv, Wo, bo, maskp, n_part):
    c32 = lambda a: np.ascontiguousarray(a, dtype=np.float32)
    c16 = lambda a: np.ascontiguousarray(a, dtype=np.float16)
    cosT = np.cos(rotary_freqs).T       # [64, LQ]
    sinT = np.sin(rotary_freqs).T
    cosf = c16(np.concatenate([cosT, cosT], axis=0))    # [128, LQ]
    ssgn = c16(np.concatenate([-sinT, sinT], axis=0))   # [128, LQ]
    in_maps = []
    for c in range(NCORES):
        b, half = c // 2, c % 2
        h0 = half * NH          # first global head
        g0 = half * NG          # first global group
        qcols = np.concatenate(
            [(h0 + h) * HD + _PERM for h in range(NH)])
        kcols = np.concatenate(
            [(g0 + g) * HD + _PERM for g in range(NG)])
        wq_c = Wq[:, qcols].reshape(KSUB, 128, NH, HD).transpose(1, 0, 2, 3)
        wk_c = Wk[:, kcols].reshape(KSUB, 128, NG, HD).transpose(1, 0, 2, 3)
        wv_c = (Wv[:, g0 * HD:(g0 + NG) * HD]
                .reshape(KSUB, 128, NG * HD).transpose(1, 0, 2))
        bqk = np.stack(
            [bq[(h0 + h) * HD + _PERM] for h in range(NH)]
            + [bk[(g0 + g) * HD + _PERM] for g in range(NG)], axis=1)
        pkt = np.stack(
            [past_k[b, g0 + g][:, _PERM].T for g in range(NG)], axis=1)
        pvt = np.stack(
            [past_v[b, g0 + g].reshape(PAST // 128, 128, HD).transpose(1, 0, 2)
             for g in range(NG)], axis=1)
        wo_c = (Wo[h0 * HD:(h0 + NH) * HD, :]
                .reshape(NH, 128, D).transpose(1, 0, 2))
        xt = x[b].T.reshape(KSUB, 128, LQ).transpose(1, 0, 2)
        m = {
            "xt": c16(xt),
            "cosf": cosf,
            "ssgn": ssgn,
            "wq": c16(wq_c),
            "bqk": c32(bqk),
            "wk": c16(wk_c),
            "wv": c16(wv_c),
            "bv": c32(bv[g0 * HD:(g0 + NG) * HD][None, :]),
            "pkt": c16(pkt),
            "pvt": c16(pvt),
            "wo": c16(wo_c),
            "bo": c32(bo[None, :] if half == 0 else np.zeros((1, D))),
        }
        if n_part:
            m["maskp"] = maskp
        in_maps.append(m)
    return in_maps


def _run(inputs, trace=False):
    from concourse import bass_utils

    classes, maskp = _classify_mask(np.asarray(inputs["mask"]))
    n_part = 0 if maskp is None else maskp.shape[1]
    key = tuple(sorted(classes.items()))
    if key not in _PROG_CACHE:
        _PROG_CACHE[key] = _build_program(classes, n_part)
    nc = _PROG_CACHE[key]

    in_maps = _prep_in_maps(
        np.asarray(inputs["x"]), np.asarray(inputs["mask"]),
        np.asarray(inputs["rotary_freqs"]), np.asarray(inputs["past_k"]),
        np.asarray(inputs["past_v"]), np.asarray(inputs["Wq"]),
        np.asarray(inputs["bq"]), np.asarray(inputs["Wk"]),
        np.asarray(inputs["bk"]), np.asarray(inputs["Wv"]),
        np.asarray(inputs["bv"]), np.asarray(inputs["Wo"]),
        np.asarray(inputs["bo"]), maskp, n_part)

    res = bass_utils.run_bass_kernel_spmd(
        nc, in_maps, list(range(NCORES)), trace=trace,
        trace_cores=list(range(NCORES)) if trace else None)

    out = np.empty((B, LQ, D), np.float32)
    for b in range(B):
        out[b] = res.results[2 * b]["out"] + res.results[2 * b + 1]["out"]
    return out, res


def kernel(**inputs) -> np.ndarray:
    out, _ = _run(inputs, trace=False)
    return out
